# revision 20
# baseline (speedup 1.0000x reference)
"""Trainium2 Bass kernel for nn_BClassifier_37546604101932 (topk_masking).

Strategy
--------
The heavy work is the deep branch over N=50000 patches:
    V = relu(feats_deep @ Wp + bp)            [N, 1024]
    s = Wc.(tanh(V@Wa+ba) * sigmoid(V@Wb+bb)) [N]      (gated-attention score)
    g = V @ Wcls                              [N]      (cls_deep dot, no bias)
That is ~99.9% of FLOPs and all of the memory traffic. It is sharded
row-block-wise over the N dimension across 8 NeuronCores (6250 rows each),
computed with bf16 matmuls (fp32 PSUM accumulate) in a single SPMD Bass
kernel with no collectives. Per-core inputs are the pre-transposed bf16
shard X^T [1024, 6250] (so the contraction dim lands on SBUF partitions
with no on-chip transpose) plus the replicated bf16 weights.

Everything downstream of (s, g) is O(N) or O(K) scalar work:
softmax/top-k/mixer/aux head. That runs on the host in fp32 numpy.
bf16 device scores carry ~7e-4 abs error while the reference's top-20
patches have internal score gaps down to 3e-5, so the top-k *ordering*
is repaired on host: the top 128 candidates by device score get their
scores recomputed exactly in fp32 (numpy matches the jax fp32 reference
to ~2e-7, far below the 1.4e-3 rank-20/21 gap), which reproduces the
reference's exact top-k selection and order.
"""

import os
import sys
from contextlib import ExitStack

import numpy as np
import ml_dtypes

# ---- problem constants (hardcoded per the task contract) ----
N, DS, DD, K = 50000, 512, 1024, 20
DATT, DAUX, HID, LAYERS = 256, 128, 128, 4
TEMP, PCT = 3.0, 0.75
NCORES = 8
MSH = N // NCORES          # 6250 rows per core
TM = 512                   # moving-dim (patch) tile: one PSUM bank of fp32
CAND = 128                 # top-k candidates re-scored exactly on host

BF16 = ml_dtypes.bfloat16

_cache = {}


def _m_tiles(msh):
    tiles = []
    off = 0
    while off < msh:
        tiles.append((off, min(TM, msh - off)))
        off += TM
    return tiles


def _build_program(msh=MSH, features=("v", "g", "ab", "s")):
    """Build + compile the per-core Bass program (cached per process)."""
    features = tuple(features)
    if ("nc", msh, features) in _cache:
        return _cache[("nc", msh, features)]

    import concourse.bacc as bacc
    import concourse.tile as tile
    from concourse import bass_isa, mybir

    f32 = mybir.dt.float32
    bf16 = mybir.dt.bfloat16
    AF = mybir.ActivationFunctionType
    KT = DD // 128    # 8 contraction tiles over DD
    DT = DATT // 128  # 2 tiles over DATT

    nc = bacc.Bacc(
        "TRN2", target_bir_lowering=False, debug=False, num_devices=NCORES
    )
    xt = nc.dram_tensor("xt", [DD, msh], bf16, kind="ExternalInput").ap()
    wp = nc.dram_tensor("wp", [DD, DD], bf16, kind="ExternalInput").ap()
    f8 = mybir.dt.float8e4
    wa = nc.dram_tensor("wa", [128, KT * DATT], f8, kind="ExternalInput").ap()
    wb = nc.dram_tensor("wb", [128, KT * DATT], f8, kind="ExternalInput").ap()
    wc = nc.dram_tensor("wc", [DATT, 1], f32, kind="ExternalInput").ap()
    wg = nc.dram_tensor("wg", [DD, 1], f32, kind="ExternalInput").ap()
    bp = nc.dram_tensor("bp", [DD, 1], f32, kind="ExternalInput").ap()
    ba = nc.dram_tensor("ba", [DATT, 1], f32, kind="ExternalInput").ap()
    bb = nc.dram_tensor("bb", [DATT, 1], f32, kind="ExternalInput").ap()
    s_out = nc.dram_tensor("s_out", [1, msh], f32, kind="ExternalOutput").ap()
    g_out = nc.dram_tensor("g_out", [1, msh], f32, kind="ExternalOutput").ap()

    with tile.TileContext(nc) as tc, ExitStack() as ctx:
        wpool = ctx.enter_context(tc.tile_pool(name="w", bufs=1))
        dpool = ctx.enter_context(tc.tile_pool(name="d", bufs=1))
        ppool = ctx.enter_context(tc.tile_pool(name="p", bufs=1, space="PSUM"))

        # --- prefetch the first m-tile's activations before the weights,
        # and issue weight DMAs on the ScalarE HWDGE queue so the two
        # streams don't serialize (head-stall fix: first MM needs xt+wp0).
        first_xts = []
        moff0, mlen0 = _m_tiles(msh)[0]
        for kt in range(KT):
            t = dpool.tile([128, mlen0], bf16, name="xt_t",
                           tag=f"xt{kt}", bufs=2)
            nc.sync.dma_start(
                t[:], xt[kt * 128:(kt + 1) * 128, moff0:moff0 + mlen0])
            first_xts.append(t)

        wa_sb = wpool.tile([128, KT, DATT], f8, name="wa_sb", tag="wa_sb")
        nc.sync.dma_start(wa_sb[:], wa[:].rearrange("p (k d) -> p k d", k=KT))
        wb_sb = wpool.tile([128, KT, DATT], f8, name="wb_sb", tag="wb_sb")
        nc.sync.dma_start(wb_sb[:], wb[:].rearrange("p (k d) -> p k d", k=KT))

        # --- resident weights/biases ---
        wp_sb, wc_sb, wg_sb = [], [], []
        bp_sb, ba_sb, bb_sb = [], [], []
        for kt in range(KT):
            t = wpool.tile([128, DD], bf16, name=f"wp{kt}", tag=f"wp{kt}")
            nc.scalar.dma_start(t[:], wp[kt * 128:(kt + 1) * 128, :])
            wp_sb.append(t)
        for kt in range(KT):
            t = wpool.tile([128, 1], f32, name=f"bp{kt}", tag=f"bp{kt}")
            nc.scalar.dma_start(t[:], bp[kt * 128:(kt + 1) * 128, :])
            bp_sb.append(t)
            t = wpool.tile([128, 1], f32, name=f"wg{kt}", tag=f"wg{kt}")
            nc.sync.dma_start(t[:], wg[kt * 128:(kt + 1) * 128, :])
            wg_sb.append(t)
        for dt_ in range(DT):
            t = wpool.tile([128, 1], f32, name=f"wc{dt_}", tag=f"wc{dt_}")
            nc.sync.dma_start(t[:], wc[dt_ * 128:(dt_ + 1) * 128, :])
            wc_sb.append(t)
            t = wpool.tile([128, 1], f32, name=f"ba{dt_}", tag=f"ba{dt_}")
            nc.sync.dma_start(t[:], ba[dt_ * 128:(dt_ + 1) * 128, :])
            ba_sb.append(t)
            t = wpool.tile([128, 1], f32, name=f"bb{dt_}", tag=f"bb{dt_}")
            nc.sync.dma_start(t[:], bb[dt_ * 128:(dt_ + 1) * 128, :])
            bb_sb.append(t)

        # bf16 copies of the (f32) wg/wc tiles for the last-tile PE path.
        # wg dram already holds Wg/16, which cancels the 16x V scale.
        wg16_sb, wc16_sb = [], []
        for kt in range(KT):
            t = wpool.tile([128, 1], bf16, name=f"wg16{kt}", tag=f"wg16{kt}")
            nc.vector.tensor_copy(t[:], wg_sb[kt][:])
            wg16_sb.append(t)
        for dt_ in range(DT):
            t = wpool.tile([128, 1], bf16, name=f"wc16{dt_}", tag=f"wc16{dt_}")
            nc.vector.tensor_copy(t[:], wc_sb[dt_][:])
            wc16_sb.append(t)

        s_sb = wpool.tile([1, msh], f32, name="s_sb", tag="s_sb")
        g_sb = wpool.tile([1, msh], f32, name="g_sb", tag="g_sb")

        if "s" not in features:
            nc.vector.memset(s_sb[:], 0.0)
        if "g" not in features:
            nc.vector.memset(g_sb[:], 0.0)

        # --- main loop over patch tiles ---
        n_mt = len(_m_tiles(msh))
        for mi, (moff, mlen) in enumerate(_m_tiles(msh)):
            is_last = mi == n_mt - 1
            if mi == 0:
                xts = first_xts
            else:
                xts = []
                for kt in range(KT):
                    t = dpool.tile([128, mlen], bf16, name="xt_t",
                                   tag=f"xt{kt}", bufs=2)
                    nc.sync.dma_start(
                        t[:], xt[kt * 128:(kt + 1) * 128, moff:moff + mlen])
                    xts.append(t)

            # V stored as one fp8 tile [128, kt, m], scaled by 16 (ACT
            # scale=16, bias pre-scaled on host) so fp8e4 stays in normal
            # range. Free stride padded to TM so DoubleRow APs stay
            # 16B-aligned on the remainder tile.
            v_all = dpool.tile([128, KT, TM], f8, name="v_all",
                               tag="v_all", bufs=2)
            for nt in range(KT):
                if "v" not in features:
                    break
                pv = ppool.tile([128, mlen], f32, name="pv", tag="pv", bufs=2)
                for kt in range(KT):
                    nc.tensor.matmul(
                        pv[:], wp_sb[kt][:, nt * 128:(nt + 1) * 128], xts[kt][:],
                        start=(kt == 0), stop=(kt == KT - 1))
                nc.scalar.activation(v_all[:, nt, :mlen], pv[:], AF.Relu,
                                     bias=bp_sb[nt][:], scale=16.0)

            if "v" not in features:
                t0 = dpool.tile([128, mlen], bf16, name="x0c", tag="x0c", bufs=2)
                nc.scalar.activation(t0[:], xts[0][:], AF.Relu)
                continue
            if "g" in features and is_last:
                # tail fix: PE is idle at the end; skip the slow gpsimd chain
                pg = ppool.tile([1, mlen], f32, name="pg", tag="pg", bufs=1)
                for kt in range(KT):
                    nc.tensor.matmul(pg[:], wg16_sb[kt][:], v_all[:, kt, :mlen],
                                     start=(kt == 0), stop=(kt == KT - 1))
                nc.scalar.copy(g_sb[0:1, moff:moff + mlen], pg[:])
            elif "g" in features:
                wv = dpool.tile([128, mlen], bf16, name="wv", tag="wv", bufs=2)
                nc.vector.tensor_scalar_mul(wv[:], v_all[:, 0, :mlen],
                                            wg_sb[0][:])
                for kt in range(1, KT):
                    nc.vector.scalar_tensor_tensor(
                        wv[:], v_all[:, kt, :mlen], wg_sb[kt][:], wv[:],
                        op0=mybir.AluOpType.mult, op1=mybir.AluOpType.add)
                gar = dpool.tile([128, mlen], f32, name="gar", tag="gar", bufs=2)
                nc.gpsimd.partition_all_reduce(
                    gar[:], wv[:], 128, bass_isa.ReduceOp.add)
                nc.scalar.copy(g_sb[0:1, moff:moff + mlen], gar[0:1, :])

            abts = []
            for dt_ in range(DT):
                if "ab" not in features:
                    break
                KP = KT // 2
                pa = ppool.tile([128, mlen], f32, name="pa", tag="pa", bufs=2)
                for kp in range(KP):
                    nc.tensor.matmul(
                        pa[:],
                        wa_sb[:, 2 * kp:2 * kp + 2,
                              dt_ * 128:(dt_ + 1) * 128],
                        v_all[:, 2 * kp:2 * kp + 2, :mlen],
                        start=(kp == 0), stop=(kp == KP - 1),
                        perf_mode=mybir.MatmulPerfMode.DoubleRow)
                pb = ppool.tile([128, mlen], f32, name="pb", tag="pb", bufs=2)
                for kp in range(KP):
                    nc.tensor.matmul(
                        pb[:],
                        wb_sb[:, 2 * kp:2 * kp + 2,
                              dt_ * 128:(dt_ + 1) * 128],
                        v_all[:, 2 * kp:2 * kp + 2, :mlen],
                        start=(kp == 0), stop=(kp == KP - 1),
                        perf_mode=mybir.MatmulPerfMode.DoubleRow)
                a_t = dpool.tile([128, mlen], bf16, name="a_t",
                                 tag=f"a{dt_}", bufs=2)
                nc.scalar.activation(a_t[:], pa[:], AF.Tanh,
                                     bias=ba_sb[dt_][:], scale=1.0 / 1024.0)
                b_t = dpool.tile([128, mlen], bf16, name="b_t",
                                 tag=f"b{dt_}", bufs=2)
                nc.scalar.activation(b_t[:], pb[:], AF.Sigmoid,
                                     bias=bb_sb[dt_][:], scale=1.0 / 1024.0)
                ab_t = dpool.tile([128, mlen], bf16, name="ab_t",
                                  tag=f"ab{dt_}", bufs=2)
                nc.vector.tensor_mul(ab_t[:], a_t[:], b_t[:])
                abts.append(ab_t)

            if "s" not in features or "ab" not in features:
                continue
            if is_last:
                ps = ppool.tile([1, mlen], f32, name="psc", tag="psc", bufs=1)
                for dt_ in range(DT):
                    nc.tensor.matmul(ps[:], wc16_sb[dt_][:], abts[dt_][:],
                                     start=(dt_ == 0), stop=(dt_ == DT - 1))
                nc.scalar.copy(s_sb[0:1, moff:moff + mlen], ps[:])
            else:
                uv = dpool.tile([128, mlen], bf16, name="uv", tag="uv", bufs=2)
                nc.vector.tensor_scalar_mul(uv[:], abts[0][:], wc_sb[0][:])
                nc.vector.scalar_tensor_tensor(
                    uv[:], abts[1][:], wc_sb[1][:], uv[:],
                    op0=mybir.AluOpType.mult, op1=mybir.AluOpType.add)
                sar = dpool.tile([128, mlen], f32, name="sar", tag="sar", bufs=2)
                nc.gpsimd.partition_all_reduce(
                    sar[:], uv[:], 128, bass_isa.ReduceOp.add)
                nc.scalar.copy(s_sb[0:1, moff:moff + mlen], sar[0:1, :])

        nc.sync.dma_start(s_out[:], s_sb[:])
        nc.sync.dma_start(g_out[:], g_sb[:])

    nc.compile()
    _cache[("nc", msh, features)] = nc
    return nc


def _maybe_install_ntff_hook():
    """Expose the axon NTFF profiling hook so trace=True yields exec times."""
    try:
        import antenv.axon_hooks  # noqa: F401
        return
    except ImportError:
        pass
    try:
        import types
        import antenv
        from trn_agent_boot.trn_boot import _ntff_profile_via_ctypes

        hook = _ntff_profile_via_ctypes("/opt/axon/libaxon_pjrt.so")
        mod = types.ModuleType("antenv.axon_hooks")
        mod._hook = hook
        mod.get_axon_ntff_profile_hook = lambda: mod._hook
        mod.set_axon_ntff_profile_hook = lambda h: setattr(mod, "_hook", h)
        sys.modules["antenv.axon_hooks"] = mod
        antenv.axon_hooks = mod
    except Exception:
        pass


LAST_EXEC_TIME_NS = None
LAST_TRACE = None


def _run_device(Xd, Wp, bp, Wa, ba, Wb, bb, Wc, Wg):
    """Run the SPMD kernel on 8 cores; returns fp32 (s[N], g[N])."""
    global LAST_EXEC_TIME_NS, LAST_TRACE

    if os.environ.get("KERNEL_SIM_HOST"):
        # numpy simulation of the device pipeline (bf16 rounding at each
        # matmul input, fp32 accumulate) — for pipeline validation only.
        F8 = ml_dtypes.float8_e4m3
        r = lambda x: x.astype(BF16).astype(np.float32)
        q8 = lambda x: x.astype(F8).astype(np.float32)
        Vq = q8(16.0 * np.maximum(r(Xd) @ r(Wp) + bp, 0.0))
        Wa8 = q8(64.0 * Wa)
        Wb8 = q8(64.0 * Wb)
        a = r(np.tanh((Vq @ Wa8) * np.float32(1 / 1024) + ba))
        b = r(1.0 / (1.0 + np.exp(-((Vq @ Wb8) * np.float32(1 / 1024) + bb))))
        ab = r(a * b)
        s = (ab @ Wc)[:, 0]
        g = ((Vq * np.float32(1 / 16)) @ Wg)[:, 0]
        return s.astype(np.float32), g.astype(np.float32)

    trace = bool(os.environ.get("BASS_TRACE"))
    if trace:
        _maybe_install_ntff_hook()

    from concourse import bass_utils

    nc = _build_program()

    F8 = ml_dtypes.float8_e4m3
    KT = DD // 128

    def _ilv8(w):
        # [DD, DATT] -> k-subtile-interleaved [128, KT*DATT] fp8, x64 scale
        q = (64.0 * w).astype(F8)
        return np.ascontiguousarray(
            q.reshape(KT, 128, DATT).transpose(1, 0, 2).reshape(128, KT * DATT))

    wp16 = np.ascontiguousarray(Wp.astype(BF16))
    wa16 = _ilv8(Wa)
    wb16 = _ilv8(Wb)
    wc16 = np.ascontiguousarray(Wc.astype(np.float32))
    wg16 = np.ascontiguousarray((Wg / 16.0).astype(np.float32))
    bp_c = np.ascontiguousarray((16.0 * bp).reshape(DD, 1).astype(np.float32))
    ba_c = np.ascontiguousarray(ba.reshape(DATT, 1).astype(np.float32))
    bb_c = np.ascontiguousarray(bb.reshape(DATT, 1).astype(np.float32))

    in_maps = []
    for c in range(NCORES):
        xt_c = np.ascontiguousarray(
            Xd[c * MSH:(c + 1) * MSH, :].astype(BF16).T)
        in_maps.append({
            "xt": xt_c, "wp": wp16, "wa": wa16, "wb": wb16,
            "wc": wc16, "wg": wg16, "bp": bp_c, "ba": ba_c, "bb": bb_c,
        })

    res = bass_utils.run_bass_kernel_spmd(
        nc, in_maps, core_ids=list(range(NCORES)), trace=trace)
    LAST_EXEC_TIME_NS = res.exec_time_ns
    LAST_TRACE = res
    s = np.concatenate([res.results[c]["s_out"][0] for c in range(NCORES)])
    g = np.concatenate([res.results[c]["g_out"][0] for c in range(NCORES)])
    return s.astype(np.float32), g.astype(np.float32)


def _erf(x):
    try:
        from scipy.special import erf
        return erf(x).astype(np.float32)
    except ImportError:
        import math
        return np.frompyfunc(math.erf, 1, 1)(
            x.astype(np.float64)).astype(np.float32)


def _gelu(x):
    return (0.5 * x * (1.0 + _erf(x * np.float32(1.0 / np.sqrt(2.0))))
            ).astype(np.float32)


def _sigmoid(x):
    return (1.0 / (1.0 + np.exp(-x))).astype(np.float32)


def kernel(feats, feats_deep, params):
    feats = np.asarray(feats, np.float32)
    Xd = np.asarray(feats_deep, np.float32)
    P = {}

    def _f32(a):
        return np.asarray(a, np.float32)

    pd = params
    Wp, bpv = _f32(pd["proj_deep"]["W"]), _f32(pd["proj_deep"]["b"])
    Wa, bav = _f32(pd["attn_deep"]["a"]["W"]), _f32(pd["attn_deep"]["a"]["b"])
    Wb, bbv = _f32(pd["attn_deep"]["b"]["W"]), _f32(pd["attn_deep"]["b"]["b"])
    Wc, bcv = _f32(pd["attn_deep"]["c"]["W"]), _f32(pd["attn_deep"]["c"]["b"])
    Wg, bgv = _f32(pd["cls_deep"]["W"]), _f32(pd["cls_deep"]["b"])
    Wcls, bcls = _f32(pd["cls"]["W"]), _f32(pd["cls"]["b"])

    # ---- device: scores s and cls-dots g for all N patches ----
    s_dev, g_dev = _run_device(Xd, Wp, bpv, Wa, bav, Wb, bbv, Wc, Wg)

    # ---- host finishing (fp32) ----
    # softmax over N (shift-invariant: the score bias bcv cancels)
    e = np.exp(s_dev - s_dev.max())
    A = (e / e.sum()).astype(np.float32)                     # [N]

    # B_deep[i] = A[i] * (V[i]@Wg) + bg ; g_dev excludes the bias
    B_deep = (A * g_dev + bgv[0]).astype(np.float32)         # [N]
    C_deep = _sigmoid(np.float32(B_deep.sum()))

    # exact top-k selection: re-score candidates in fp32
    cand = np.argpartition(-s_dev, CAND)[:CAND]
    Xc = Xd[cand]
    Vc = np.maximum(Xc @ Wp + bpv, 0.0)
    ac = np.tanh(Vc @ Wa + bav)
    bc_ = _sigmoid(Vc @ Wb + bbv)
    sc = ((ac * bc_) @ Wc)[:, 0] + bcv[0]
    # stable descending order with index tiebreak (matches jnp.argsort(-s))
    ordc = np.lexsort((cand, -sc))
    topk_idx = cand[ordc[:K]]                                # [K]

    topk_feats = feats[topk_idx]                             # [K, DS]
    x = np.ascontiguousarray(topk_feats.T)[None]             # [1, DS, K]

    for lyr in params["mixer"]:
        t1W, t1b = _f32(lyr["tok1"]["W"]), _f32(lyr["tok1"]["b"])
        t2W, t2b = _f32(lyr["tok2"]["W"]), _f32(lyr["tok2"]["b"])
        c1W, c1b = _f32(lyr["ch1"]["W"]), _f32(lyr["ch1"]["b"])
        c2W, c2b = _f32(lyr["ch2"]["W"]), _f32(lyr["ch2"]["b"])
        t = np.swapaxes(x, 1, 2)                             # [1, K, DS]
        t = _gelu(t @ t1W + t1b) @ t2W + t2b
        x = x + np.swapaxes(t, 1, 2)
        x = x + (_gelu(x @ c1W + c1b) @ c2W + c2b)
    x = x.astype(np.float32)

    ga = params["aux_ga"]
    aW, ab_ = _f32(ga["a"]["W"]), _f32(ga["a"]["b"])
    bW, bb_ = _f32(ga["b"]["W"]), _f32(ga["b"]["b"])
    cW, cb_ = _f32(ga["c"]["W"]), _f32(ga["c"]["b"])
    aa = np.tanh(x @ aW + ab_)
    bbx = _sigmoid(x @ bW + bb_)
    A_aux = ((aa * bbx) @ cW + cb_)[..., 0]                  # [1, DS]

    q = np.quantile(A_aux.astype(np.float64), PCT, axis=1,
                    keepdims=True).astype(np.float32)
    sd = np.std(A_aux, axis=1, keepdims=True, ddof=1).astype(np.float32)
    A_aux = _sigmoid((A_aux - q) / (sd + np.float32(1e-6)) * np.float32(TEMP))

    Bmat = ((topk_feats * A_aux) @ Wcls + bcls).astype(np.float32)  # [K, 1]
    C = _sigmoid(Bmat.sum(0).astype(np.float32))                     # [1]

    return (
        np.asarray(C, np.float32).reshape(1),
        np.asarray(C_deep, np.float32).reshape(1),
        A.reshape(N, 1).astype(np.float32),
        Bmat.astype(np.float32),
        B_deep.reshape(N, 1).astype(np.float32),
        A_aux.astype(np.float32),
    )


# revision 21
# speedup vs baseline: 1.1882x; 1.1882x over previous
"""Trainium2 Bass kernel for nn_BClassifier_37546604101932 (topk_masking).

Strategy
--------
The heavy work is the deep branch over N=50000 patches:
    V = relu(feats_deep @ Wp + bp)            [N, 1024]
    s = Wc.(tanh(V@Wa+ba) * sigmoid(V@Wb+bb)) [N]      (gated-attention score)
    g = V @ Wcls                              [N]      (cls_deep dot, no bias)
That is ~99.9% of FLOPs and all of the memory traffic. It is sharded
row-block-wise over the N dimension across 8 NeuronCores (6250 rows each),
computed with bf16 matmuls (fp32 PSUM accumulate) in a single SPMD Bass
kernel with no collectives. Per-core inputs are the pre-transposed bf16
shard X^T [1024, 6250] (so the contraction dim lands on SBUF partitions
with no on-chip transpose) plus the replicated bf16 weights.

Everything downstream of (s, g) is O(N) or O(K) scalar work:
softmax/top-k/mixer/aux head. That runs on the host in fp32 numpy.
bf16 device scores carry ~7e-4 abs error while the reference's top-20
patches have internal score gaps down to 3e-5, so the top-k *ordering*
is repaired on host: the top 128 candidates by device score get their
scores recomputed exactly in fp32 (numpy matches the jax fp32 reference
to ~2e-7, far below the 1.4e-3 rank-20/21 gap), which reproduces the
reference's exact top-k selection and order.
"""

import os
import sys
from contextlib import ExitStack

import numpy as np
import ml_dtypes

# ---- problem constants (hardcoded per the task contract) ----
N, DS, DD, K = 50000, 512, 1024, 20
DATT, DAUX, HID, LAYERS = 256, 128, 128, 4
TEMP, PCT = 3.0, 0.75
NCORES = 8
MSH = N // NCORES          # 6250 rows per core
TM = 512                   # moving-dim (patch) tile: one PSUM bank of fp32
CAND = 128                 # top-k candidates re-scored exactly on host

BF16 = ml_dtypes.bfloat16

_cache = {}


def _m_tiles(msh):
    tiles = []
    off = 0
    while off < msh:
        tiles.append((off, min(TM, msh - off)))
        off += TM
    return tiles


def _build_program(msh=MSH, features=("v", "g", "ab", "s")):
    """Build + compile the per-core Bass program (cached per process)."""
    features = tuple(features)
    if ("nc", msh, features) in _cache:
        return _cache[("nc", msh, features)]

    import concourse.bacc as bacc
    import concourse.tile as tile
    from concourse import bass_isa, mybir

    f32 = mybir.dt.float32
    bf16 = mybir.dt.bfloat16
    AF = mybir.ActivationFunctionType
    KT = DD // 128    # 8 contraction tiles over DD
    DT = DATT // 128  # 2 tiles over DATT

    nc = bacc.Bacc(
        "TRN2", target_bir_lowering=False, debug=False, num_devices=NCORES
    )
    xt = nc.dram_tensor("xt", [DD, msh], bf16, kind="ExternalInput").ap()
    wp = nc.dram_tensor("wp", [DD, DD], bf16, kind="ExternalInput").ap()
    f8 = mybir.dt.float8e4
    wa = nc.dram_tensor("wa", [128, KT * DATT], f8, kind="ExternalInput").ap()
    wb = nc.dram_tensor("wb", [128, KT * DATT], f8, kind="ExternalInput").ap()
    wc = nc.dram_tensor("wc", [DATT, 1], f32, kind="ExternalInput").ap()
    wg = nc.dram_tensor("wg", [DD, 1], f32, kind="ExternalInput").ap()
    bp = nc.dram_tensor("bp", [DD, 1], f32, kind="ExternalInput").ap()
    ba = nc.dram_tensor("ba", [DATT, 1], f32, kind="ExternalInput").ap()
    bb = nc.dram_tensor("bb", [DATT, 1], f32, kind="ExternalInput").ap()
    s_out = nc.dram_tensor("s_out", [1, msh], f32, kind="ExternalOutput").ap()
    g_out = nc.dram_tensor("g_out", [1, msh], f32, kind="ExternalOutput").ap()

    with tile.TileContext(nc) as tc, ExitStack() as ctx:
        wpool = ctx.enter_context(tc.tile_pool(name="w", bufs=1))
        dpool = ctx.enter_context(tc.tile_pool(name="d", bufs=1))
        ppool = ctx.enter_context(tc.tile_pool(name="p", bufs=1, space="PSUM"))

        # --- prefetch the first m-tile's activations before the weights,
        # and issue weight DMAs on the ScalarE HWDGE queue so the two
        # streams don't serialize (head-stall fix: first MM needs xt+wp0).
        first_xts = []
        moff0, mlen0 = _m_tiles(msh)[0]
        for kt in range(KT):
            t = dpool.tile([128, mlen0], bf16, name="xt_t",
                           tag=f"xt{kt}", bufs=2)
            nc.sync.dma_start(
                t[:], xt[kt * 128:(kt + 1) * 128, moff0:moff0 + mlen0])
            first_xts.append(t)

        wa_sb = wpool.tile([128, KT, DATT], f8, name="wa_sb", tag="wa_sb")
        nc.sync.dma_start(wa_sb[:], wa[:].rearrange("p (k d) -> p k d", k=KT))
        wb_sb = wpool.tile([128, KT, DATT], f8, name="wb_sb", tag="wb_sb")
        nc.sync.dma_start(wb_sb[:], wb[:].rearrange("p (k d) -> p k d", k=KT))

        # --- resident weights/biases ---
        wp_sb, wc_sb, wg_sb = [], [], []
        bp_sb, ba_sb, bb_sb = [], [], []
        for kt in range(KT):
            t = wpool.tile([128, DD], bf16, name=f"wp{kt}", tag=f"wp{kt}")
            nc.scalar.dma_start(t[:], wp[kt * 128:(kt + 1) * 128, :])
            wp_sb.append(t)
        for kt in range(KT):
            t = wpool.tile([128, 1], f32, name=f"bp{kt}", tag=f"bp{kt}")
            nc.scalar.dma_start(t[:], bp[kt * 128:(kt + 1) * 128, :])
            bp_sb.append(t)
            t = wpool.tile([128, 1], f32, name=f"wg{kt}", tag=f"wg{kt}")
            nc.sync.dma_start(t[:], wg[kt * 128:(kt + 1) * 128, :])
            wg_sb.append(t)
        for dt_ in range(DT):
            t = wpool.tile([128, 1], f32, name=f"wc{dt_}", tag=f"wc{dt_}")
            nc.sync.dma_start(t[:], wc[dt_ * 128:(dt_ + 1) * 128, :])
            wc_sb.append(t)
            t = wpool.tile([128, 1], f32, name=f"ba{dt_}", tag=f"ba{dt_}")
            nc.sync.dma_start(t[:], ba[dt_ * 128:(dt_ + 1) * 128, :])
            ba_sb.append(t)
            t = wpool.tile([128, 1], f32, name=f"bb{dt_}", tag=f"bb{dt_}")
            nc.sync.dma_start(t[:], bb[dt_ * 128:(dt_ + 1) * 128, :])
            bb_sb.append(t)

        # bf16 copies of the (f32) wg/wc tiles for the last-tile PE path.
        # wg dram already holds Wg/16, which cancels the 16x V scale.
        wg16_sb, wc16_sb = [], []
        for kt in range(KT):
            t = wpool.tile([128, 1], bf16, name=f"wg16{kt}", tag=f"wg16{kt}")
            nc.vector.tensor_copy(t[:], wg_sb[kt][:])
            wg16_sb.append(t)
        for dt_ in range(DT):
            t = wpool.tile([128, 1], bf16, name=f"wc16{dt_}", tag=f"wc16{dt_}")
            nc.vector.tensor_copy(t[:], wc_sb[dt_][:])
            wc16_sb.append(t)

        s_sb = wpool.tile([1, msh], f32, name="s_sb", tag="s_sb")
        g_sb = wpool.tile([1, msh], f32, name="g_sb", tag="g_sb")

        if "s" not in features:
            nc.vector.memset(s_sb[:], 0.0)
        if "g" not in features:
            nc.vector.memset(g_sb[:], 0.0)

        # --- main loop over patch tiles ---
        n_mt = len(_m_tiles(msh))
        for mi, (moff, mlen) in enumerate(_m_tiles(msh)):
            is_last = mi == n_mt - 1
            if mi == 0:
                xts = first_xts
            else:
                xts = []
                for kt in range(KT):
                    t = dpool.tile([128, mlen], bf16, name="xt_t",
                                   tag=f"xt{kt}", bufs=2)
                    nc.sync.dma_start(
                        t[:], xt[kt * 128:(kt + 1) * 128, moff:moff + mlen])
                    xts.append(t)

            # V stored as one fp8 tile [128, kt, m], scaled by 16 (ACT
            # scale=16, bias pre-scaled on host) so fp8e4 stays in normal
            # range. Free stride padded to TM so DoubleRow APs stay
            # 16B-aligned on the remainder tile.
            v_all = dpool.tile([128, KT, TM], f8, name="v_all",
                               tag="v_all", bufs=2)
            for nt in range(KT):
                if "v" not in features:
                    break
                pv = ppool.tile([128, mlen], f32, name="pv", tag="pv", bufs=2)
                for kt in range(KT):
                    nc.tensor.matmul(
                        pv[:], wp_sb[kt][:, nt * 128:(nt + 1) * 128], xts[kt][:],
                        start=(kt == 0), stop=(kt == KT - 1))
                nc.scalar.activation(v_all[:, nt, :mlen], pv[:], AF.Relu,
                                     bias=bp_sb[nt][:], scale=16.0)

            if "v" not in features:
                t0 = dpool.tile([128, mlen], bf16, name="x0c", tag="x0c", bufs=2)
                nc.scalar.activation(t0[:], xts[0][:], AF.Relu)
                continue
            if "g" in features and is_last:
                # tail fix: PE is idle at the end; skip the slow gpsimd chain
                pg = ppool.tile([1, mlen], f32, name="pg", tag="pg", bufs=1)
                for kt in range(KT):
                    nc.tensor.matmul(pg[:], wg16_sb[kt][:], v_all[:, kt, :mlen],
                                     start=(kt == 0), stop=(kt == KT - 1))
                nc.scalar.copy(g_sb[0:1, moff:moff + mlen], pg[:])
            elif "g" in features:
                wv = dpool.tile([128, mlen], bf16, name="wv", tag="wv", bufs=2)
                nc.vector.tensor_scalar_mul(wv[:], v_all[:, 0, :mlen],
                                            wg_sb[0][:])
                for kt in range(1, KT):
                    nc.vector.scalar_tensor_tensor(
                        wv[:], v_all[:, kt, :mlen], wg_sb[kt][:], wv[:],
                        op0=mybir.AluOpType.mult, op1=mybir.AluOpType.add)
                gar = dpool.tile([128, mlen], f32, name="gar", tag="gar", bufs=2)
                nc.gpsimd.partition_all_reduce(
                    gar[:], wv[:], 128, bass_isa.ReduceOp.add)
                nc.vector.tensor_copy(g_sb[0:1, moff:moff + mlen], gar[0:1, :])

            abts = []
            for dt_ in range(DT):
                if "ab" not in features:
                    break
                KP = KT // 2
                pa = ppool.tile([128, mlen], f32, name="pa", tag="pa", bufs=2)
                for kp in range(KP):
                    nc.tensor.matmul(
                        pa[:],
                        wa_sb[:, 2 * kp:2 * kp + 2,
                              dt_ * 128:(dt_ + 1) * 128],
                        v_all[:, 2 * kp:2 * kp + 2, :mlen],
                        start=(kp == 0), stop=(kp == KP - 1),
                        perf_mode=mybir.MatmulPerfMode.DoubleRow)
                pb = ppool.tile([128, mlen], f32, name="pb", tag="pb", bufs=2)
                for kp in range(KP):
                    nc.tensor.matmul(
                        pb[:],
                        wb_sb[:, 2 * kp:2 * kp + 2,
                              dt_ * 128:(dt_ + 1) * 128],
                        v_all[:, 2 * kp:2 * kp + 2, :mlen],
                        start=(kp == 0), stop=(kp == KP - 1),
                        perf_mode=mybir.MatmulPerfMode.DoubleRow)
                a_t = dpool.tile([128, mlen], bf16, name="a_t",
                                 tag=f"a{dt_}", bufs=2)
                nc.scalar.activation(a_t[:], pa[:], AF.Tanh,
                                     bias=ba_sb[dt_][:], scale=1.0 / 1024.0)
                b_t = dpool.tile([128, mlen], bf16, name="b_t",
                                 tag=f"b{dt_}", bufs=2)
                nc.scalar.activation(b_t[:], pb[:], AF.Sigmoid,
                                     bias=bb_sb[dt_][:], scale=1.0 / 1024.0)
                ab_t = dpool.tile([128, mlen], bf16, name="ab_t",
                                  tag=f"ab{dt_}", bufs=2)
                nc.vector.tensor_mul(ab_t[:], a_t[:], b_t[:])
                abts.append(ab_t)

            if "s" not in features or "ab" not in features:
                continue
            if is_last:
                ps = ppool.tile([1, mlen], f32, name="psc", tag="psc", bufs=1)
                for dt_ in range(DT):
                    nc.tensor.matmul(ps[:], wc16_sb[dt_][:], abts[dt_][:],
                                     start=(dt_ == 0), stop=(dt_ == DT - 1))
                nc.scalar.copy(s_sb[0:1, moff:moff + mlen], ps[:])
            else:
                uv = dpool.tile([128, mlen], bf16, name="uv", tag="uv", bufs=2)
                nc.vector.tensor_scalar_mul(uv[:], abts[0][:], wc_sb[0][:])
                nc.vector.scalar_tensor_tensor(
                    uv[:], abts[1][:], wc_sb[1][:], uv[:],
                    op0=mybir.AluOpType.mult, op1=mybir.AluOpType.add)
                sar = dpool.tile([128, mlen], f32, name="sar", tag="sar", bufs=2)
                nc.gpsimd.partition_all_reduce(
                    sar[:], uv[:], 128, bass_isa.ReduceOp.add)
                nc.vector.tensor_copy(s_sb[0:1, moff:moff + mlen], sar[0:1, :])

        nc.sync.dma_start(s_out[:], s_sb[:])
        nc.sync.dma_start(g_out[:], g_sb[:])

    nc.compile()
    _cache[("nc", msh, features)] = nc
    return nc


def _maybe_install_ntff_hook():
    """Expose the axon NTFF profiling hook so trace=True yields exec times."""
    try:
        import antenv.axon_hooks  # noqa: F401
        return
    except ImportError:
        pass
    try:
        import types
        import antenv
        from trn_agent_boot.trn_boot import _ntff_profile_via_ctypes

        hook = _ntff_profile_via_ctypes("/opt/axon/libaxon_pjrt.so")
        mod = types.ModuleType("antenv.axon_hooks")
        mod._hook = hook
        mod.get_axon_ntff_profile_hook = lambda: mod._hook
        mod.set_axon_ntff_profile_hook = lambda h: setattr(mod, "_hook", h)
        sys.modules["antenv.axon_hooks"] = mod
        antenv.axon_hooks = mod
    except Exception:
        pass


LAST_EXEC_TIME_NS = None
LAST_TRACE = None


def _run_device(Xd, Wp, bp, Wa, ba, Wb, bb, Wc, Wg):
    """Run the SPMD kernel on 8 cores; returns fp32 (s[N], g[N])."""
    global LAST_EXEC_TIME_NS, LAST_TRACE

    if os.environ.get("KERNEL_SIM_HOST"):
        # numpy simulation of the device pipeline (bf16 rounding at each
        # matmul input, fp32 accumulate) — for pipeline validation only.
        F8 = ml_dtypes.float8_e4m3
        r = lambda x: x.astype(BF16).astype(np.float32)
        q8 = lambda x: x.astype(F8).astype(np.float32)
        Vq = q8(16.0 * np.maximum(r(Xd) @ r(Wp) + bp, 0.0))
        Wa8 = q8(64.0 * Wa)
        Wb8 = q8(64.0 * Wb)
        a = r(np.tanh((Vq @ Wa8) * np.float32(1 / 1024) + ba))
        b = r(1.0 / (1.0 + np.exp(-((Vq @ Wb8) * np.float32(1 / 1024) + bb))))
        ab = r(a * b)
        s = (ab @ Wc)[:, 0]
        g = ((Vq * np.float32(1 / 16)) @ Wg)[:, 0]
        return s.astype(np.float32), g.astype(np.float32)

    trace = bool(os.environ.get("BASS_TRACE"))
    if trace:
        _maybe_install_ntff_hook()

    from concourse import bass_utils

    nc = _build_program()

    F8 = ml_dtypes.float8_e4m3
    KT = DD // 128

    def _ilv8(w):
        # [DD, DATT] -> k-subtile-interleaved [128, KT*DATT] fp8, x64 scale
        q = (64.0 * w).astype(F8)
        return np.ascontiguousarray(
            q.reshape(KT, 128, DATT).transpose(1, 0, 2).reshape(128, KT * DATT))

    wp16 = np.ascontiguousarray(Wp.astype(BF16))
    wa16 = _ilv8(Wa)
    wb16 = _ilv8(Wb)
    wc16 = np.ascontiguousarray(Wc.astype(np.float32))
    wg16 = np.ascontiguousarray((Wg / 16.0).astype(np.float32))
    bp_c = np.ascontiguousarray((16.0 * bp).reshape(DD, 1).astype(np.float32))
    ba_c = np.ascontiguousarray(ba.reshape(DATT, 1).astype(np.float32))
    bb_c = np.ascontiguousarray(bb.reshape(DATT, 1).astype(np.float32))

    in_maps = []
    for c in range(NCORES):
        xt_c = np.ascontiguousarray(
            Xd[c * MSH:(c + 1) * MSH, :].astype(BF16).T)
        in_maps.append({
            "xt": xt_c, "wp": wp16, "wa": wa16, "wb": wb16,
            "wc": wc16, "wg": wg16, "bp": bp_c, "ba": ba_c, "bb": bb_c,
        })

    res = bass_utils.run_bass_kernel_spmd(
        nc, in_maps, core_ids=list(range(NCORES)), trace=trace)
    LAST_EXEC_TIME_NS = res.exec_time_ns
    LAST_TRACE = res
    s = np.concatenate([res.results[c]["s_out"][0] for c in range(NCORES)])
    g = np.concatenate([res.results[c]["g_out"][0] for c in range(NCORES)])
    return s.astype(np.float32), g.astype(np.float32)


def _erf(x):
    try:
        from scipy.special import erf
        return erf(x).astype(np.float32)
    except ImportError:
        import math
        return np.frompyfunc(math.erf, 1, 1)(
            x.astype(np.float64)).astype(np.float32)


def _gelu(x):
    return (0.5 * x * (1.0 + _erf(x * np.float32(1.0 / np.sqrt(2.0))))
            ).astype(np.float32)


def _sigmoid(x):
    return (1.0 / (1.0 + np.exp(-x))).astype(np.float32)


def kernel(feats, feats_deep, params):
    feats = np.asarray(feats, np.float32)
    Xd = np.asarray(feats_deep, np.float32)
    P = {}

    def _f32(a):
        return np.asarray(a, np.float32)

    pd = params
    Wp, bpv = _f32(pd["proj_deep"]["W"]), _f32(pd["proj_deep"]["b"])
    Wa, bav = _f32(pd["attn_deep"]["a"]["W"]), _f32(pd["attn_deep"]["a"]["b"])
    Wb, bbv = _f32(pd["attn_deep"]["b"]["W"]), _f32(pd["attn_deep"]["b"]["b"])
    Wc, bcv = _f32(pd["attn_deep"]["c"]["W"]), _f32(pd["attn_deep"]["c"]["b"])
    Wg, bgv = _f32(pd["cls_deep"]["W"]), _f32(pd["cls_deep"]["b"])
    Wcls, bcls = _f32(pd["cls"]["W"]), _f32(pd["cls"]["b"])

    # ---- device: scores s and cls-dots g for all N patches ----
    s_dev, g_dev = _run_device(Xd, Wp, bpv, Wa, bav, Wb, bbv, Wc, Wg)

    # ---- host finishing (fp32) ----
    # softmax over N (shift-invariant: the score bias bcv cancels)
    e = np.exp(s_dev - s_dev.max())
    A = (e / e.sum()).astype(np.float32)                     # [N]

    # B_deep[i] = A[i] * (V[i]@Wg) + bg ; g_dev excludes the bias
    B_deep = (A * g_dev + bgv[0]).astype(np.float32)         # [N]
    C_deep = _sigmoid(np.float32(B_deep.sum()))

    # exact top-k selection: re-score candidates in fp32
    cand = np.argpartition(-s_dev, CAND)[:CAND]
    Xc = Xd[cand]
    Vc = np.maximum(Xc @ Wp + bpv, 0.0)
    ac = np.tanh(Vc @ Wa + bav)
    bc_ = _sigmoid(Vc @ Wb + bbv)
    sc = ((ac * bc_) @ Wc)[:, 0] + bcv[0]
    # stable descending order with index tiebreak (matches jnp.argsort(-s))
    ordc = np.lexsort((cand, -sc))
    topk_idx = cand[ordc[:K]]                                # [K]

    topk_feats = feats[topk_idx]                             # [K, DS]
    x = np.ascontiguousarray(topk_feats.T)[None]             # [1, DS, K]

    for lyr in params["mixer"]:
        t1W, t1b = _f32(lyr["tok1"]["W"]), _f32(lyr["tok1"]["b"])
        t2W, t2b = _f32(lyr["tok2"]["W"]), _f32(lyr["tok2"]["b"])
        c1W, c1b = _f32(lyr["ch1"]["W"]), _f32(lyr["ch1"]["b"])
        c2W, c2b = _f32(lyr["ch2"]["W"]), _f32(lyr["ch2"]["b"])
        t = np.swapaxes(x, 1, 2)                             # [1, K, DS]
        t = _gelu(t @ t1W + t1b) @ t2W + t2b
        x = x + np.swapaxes(t, 1, 2)
        x = x + (_gelu(x @ c1W + c1b) @ c2W + c2b)
    x = x.astype(np.float32)

    ga = params["aux_ga"]
    aW, ab_ = _f32(ga["a"]["W"]), _f32(ga["a"]["b"])
    bW, bb_ = _f32(ga["b"]["W"]), _f32(ga["b"]["b"])
    cW, cb_ = _f32(ga["c"]["W"]), _f32(ga["c"]["b"])
    aa = np.tanh(x @ aW + ab_)
    bbx = _sigmoid(x @ bW + bb_)
    A_aux = ((aa * bbx) @ cW + cb_)[..., 0]                  # [1, DS]

    q = np.quantile(A_aux.astype(np.float64), PCT, axis=1,
                    keepdims=True).astype(np.float32)
    sd = np.std(A_aux, axis=1, keepdims=True, ddof=1).astype(np.float32)
    A_aux = _sigmoid((A_aux - q) / (sd + np.float32(1e-6)) * np.float32(TEMP))

    Bmat = ((topk_feats * A_aux) @ Wcls + bcls).astype(np.float32)  # [K, 1]
    C = _sigmoid(Bmat.sum(0).astype(np.float32))                     # [1]

    return (
        np.asarray(C, np.float32).reshape(1),
        np.asarray(C_deep, np.float32).reshape(1),
        A.reshape(N, 1).astype(np.float32),
        Bmat.astype(np.float32),
        B_deep.reshape(N, 1).astype(np.float32),
        A_aux.astype(np.float32),
    )
